# revision 26
# baseline (speedup 1.0000x reference)
"""Trainium2 Bass kernel for nn_Attn_6545530159401.

Computation (reference):
    enc  = encoder_outputs.transpose(1,0,2)            # (B,T,H)
    cat  = concat([hidden broadcast, enc], -1)         # (B,T,2H)
    en   = tanh(cat @ W_attn.T + b_attn)               # (B,T,H)
    sc   = en @ v                                      # (B,T)
    out  = softmax(sc, axis=1)[:, None, :]             # (B,1,T)

Split W_attn = [W_h | W_e] (each (H,H)):
    q[b]     = hidden[b] @ W_h.T + b_attn              # computed on HOST
    E[b,t]   = enc[b,t] @ W_e.T                        # the big matmul
    sc[b,t]  = sum_o v[o] * tanh(q[b,o] + E[b,t,o])

Sharding: data-parallel over B across 8 NeuronCores (4 batches/core),
no collectives. q is a (B, H) matvec on the inputs — done exactly on
the host and shipped inside the consts block, so the device has no
q path at all. Per-core pipeline (o-chunks on PSUM partitions so q
rides the ACT bias port):
    mains   PE: E-psum (128 o, 1024 rows) accumulated over 2 k-pairs as
            fp8e4 DoubleRow matmuls (256-deep contraction each, 2x the
            bf16 MAC rate; enc scaled x16, W_e x32 on the host, undone
            by the tanh ACTIVATE's scale=1/512); kp-outer/h2-inner so
            each LDWEIGHTS feeds two N=512 matmuls
    tanh    ACT: tanh(E/512 + q[b, o-chunk]) via per-partition bias
    z       GPSIMD seeds zB = v_0 * tanh_0, then DVE chains the three
            remaining o-chunks with fused scalar_tensor_tensor
            (balances DVE against the ACT 4us/superblock cadence)
    scores  PE: sliding-window ones lhsT contracts z's 128 partitions
            into row 2s+h2 of one persistent (16, 512) psum tile
            (last superblock v-dots straight from tanh so the tail never
            waits on the DVE chain); all emissions pipelined one
            iteration behind so the in-order PE never waits on ACT/DVE
    softmax ACT exp with fused accum sum on (16, 512), per-batch
            sum/broadcast via tiny PE matmuls, no max-subtraction
            (scores are bounded by ||v||_1)
Tiny-N warmup matmuls run inside the head-DMA shadow so the PE clock
gate (HAM) releases before the first real matmuls; head DMAs are split
across the two HWDGE rings and ordered by what the PE needs first.
"""

import numpy as np
import ml_dtypes
from contextlib import ExitStack

import concourse.bass as bass
import concourse.tile as tile
from concourse import bacc, mybir
import concourse.bass_utils as bass_utils

T, B, H = 2048, 32, 512
NCORES = 8
NB = B // NCORES        # 4 local batches per core
ROWS = NB * T           # 8192 rows per core
P = 128
KC = H // P             # 4 contraction chunks
KP = KC // 2            # 2 DoubleRow k-pairs (256-deep each)
OC = H // P             # 4 output chunks
SUP = 1024              # columns per E-psum tile (2 PSUM banks)
NSUP = ROWS // SUP      # 8
BLK = 512               # matmul moving-dim limit (one PSUM bank)
SE = 16.0               # host-side fp8 scale on enc
SW = 32.0               # host-side fp8 scale on W_e
F32 = mybir.dt.float32
F32R = mybir.dt.float32r
F16 = mybir.dt.float16
F8 = mybir.dt.float8e4
NP8 = ml_dtypes.float8_e4m3
DR = mybir.MatmulPerfMode.DoubleRow
AF = mybir.ActivationFunctionType


def _build():
    nc = bacc.Bacc(
        "TRN2", target_bir_lowering=False, debug=False, num_devices=NCORES
    )
    # enct is packed k-chunk-major: enct[p, k, r] = fp8(SE * enc^T[k*128+p, r])
    enct = nc.declare_dram_parameter("enct", [P, KC, ROWS], F8, isOutput=False)
    # wetp packs the 4 k-chunks side by side: [p, k, o] = fp8(SW * W_e^T[k*128+p, o])
    wetp = nc.declare_dram_parameter("wetp", [P, KC, H], F8, isOutput=False)
    # o=0 slice of wetp: wet0[p, k, o] = fp8(SW * W_e^T[k*128+p, o]) (o<128)
    wet0 = nc.declare_dram_parameter("wet0", [P, KC, P], F8, isOutput=False)
    # all small constants in one (P, 128) f32 block (f16 regions bitcast):
    #   f32 cols  0:16  qrep[p, o*NB+b] = q[b0+b, o*128+p]  (host-computed)
    #   f32 cols 16:20  vp[p, o] = v[o*128+p]
    #   f32 cols 20:24  selb2 (rows 0:16)
    #   f32 cols 24:40  selb  (rows 0:4)
    #   f32 cols 40:56  onesw as f16[32]: ones at f16-col 15 (sliding lhsT)
    #   f32 cols 64:128 vwin: 4 chunks of 32 f16 cols, chunk o has
    #                   v[o*128+p] at f16-col 128+32*o+15 (sliding lhsT)
    consts = nc.declare_dram_parameter("consts", [P, 128], F32, isOutput=False)
    out = nc.declare_dram_parameter("out", [NB, T], F32, isOutput=True)

    with tile.TileContext(nc) as tc, ExitStack() as ctx:
        const_pool = ctx.enter_context(tc.tile_pool(name="const", bufs=1))
        enc_pool = ctx.enter_context(tc.tile_pool(name="enc", bufs=1))
        tanh_pool = ctx.enter_context(tc.tile_pool(name="tanh", bufs=6))
        sm_pool = ctx.enter_context(tc.tile_pool(name="sm", bufs=1))
        psE_pool = ctx.enter_context(tc.tile_pool(name="psE", bufs=3, space="PSUM"))
        psS_pool = ctx.enter_context(tc.tile_pool(name="psS", bufs=1, space="PSUM"))

        # o=0 weight slice lands first (small), in parallel (scalar ring)
        # with the k-chunks of the first enc superblock (sync ring), so the
        # first main matmuls can start as early as possible.
        wet0_sb = const_pool.tile([P, KC, P], F8, tag="wet0")
        nc.scalar.dma_start(wet0_sb[:], wet0[:, :, :])

        def load_sup(s, engine=None):
            e = enc_pool.tile([P, KC, SUP], F8, tag=f"e{s}", name=f"e{s}")
            (engine or nc.sync).dma_start(
                e[:], enct[:, :, s * SUP : (s + 1) * SUP]
            )
            return e

        # whole enc shard stays SBUF-resident (4 * 4KB/partition = 16KB);
        # one 512KB DMA per superblock. The first superblock is split per
        # k-chunk across BOTH rings (kp0 on sync, kp1 on scalar) so both
        # DoubleRow k-pairs land as early as possible.
        enc_sb = [None] * NSUP
        e0 = enc_pool.tile([P, KC, SUP], F8, tag="e0", name="e0")
        # k-pair 0 chunks on the sync ring, k-pair 1 on the scalar ring
        # (behind wet0): per-DMA latency is ~1.2us serialized per ring,
        # so splitting gets both DoubleRow k-pairs in ~2 DMA slots
        for k in range(2):
            nc.sync.dma_start(e0[:, k, :], enct[:, k, 0:SUP])
        for k in range(2, KC):
            nc.scalar.dma_start(e0[:, k, :], enct[:, k, 0:SUP])
        enc_sb[0] = e0

        c_sb = const_pool.tile([P, 128], F32, tag="consts")
        nc.scalar.dma_start(c_sb[:], consts[:, :])
        c16 = c_sb[:].bitcast(F16)  # (P, 256) f16 view
        q_sb = c_sb[:, 0:16]
        vp_sb = c_sb[:, 16:20]
        selb2_sb = c_sb[0:16, 20:24]
        selb_sb = c_sb[0:4, 24:40]
        onesw_sb = c16[:, 80:111]
        vwin_sb = [c16[:, 128 + 32 * o : 159 + 32 * o] for o in range(OC)]

        wetp_sb = const_pool.tile([P, KC, H], F8, tag="wetp")
        nc.scalar.dma_start(wetp_sb[:], wetp[:, :, :])

        # late superblocks ride the scalar ring (it is idle after the
        # constant loads), halving the enc DMA tail on the sync ring
        for s in range(1, NSUP):
            enc_sb[s] = load_sup(
                s, engine=nc.scalar if s >= NSUP // 2 else nc.sync
            )

        # PE warmup: tiny N=64 matmuls in the head-DMA shadow keep the PE
        # queue primed; the first (DMA-paced) real superblocks then warm
        # the HAM clock gate.
        warm = const_pool.tile([P, 64], F16, tag="warm")
        nc.gpsimd.memset(warm[:], 0.0)
        psW = psS_pool.tile([P, 64], F32, tag="t", name="psW")
        for _ in range(8):
            nc.tensor.matmul(
                psW[0:64, :], lhsT=warm[:, 0:64], rhs=warm[:],
                start=True, stop=True,
            )

        # single persistent score accumulator: row 2s+h2 = scores of
        # (batch s//2, t-slice (s%2)*1024 + h2*512)
        psS16 = psS_pool.tile([16, BLK], F32, tag="s16", name="psS16")

        # z(s) = sum_o v_o * tanh(E_o + q_o) is built on DVE (fused
        # scalar_tensor_tensor), then one small matmul per 512-block
        # contracts the 128 partitions into psS. The z-matmul for s is
        # emitted during s+1's main matmuls so the in-order PE never
        # waits on the DVE chain it just scheduled.
        def emit_zmm(z, s, b):
            for h2 in range(SUP // BLK):
                c = 2 * s + h2
                nc.tensor.matmul(
                    psS16[:],
                    lhsT=onesw_sb[:, 15 - c : 31 - c],
                    rhs=z[:, h2 * BLK : (h2 + 1) * BLK],
                    start=(s == 0 and h2 == 0),
                    stop=(s == NSUP - 1 and h2 == 1),
                )

        def emit_vdot(th, s, o):
            # direct PE v-dot (used for the last superblock so the tail
            # does not wait on the DVE z-chain)
            for h2 in range(SUP // BLK):
                c = 2 * s + h2
                nc.tensor.matmul(
                    psS16[:],
                    lhsT=vwin_sb[o][:, 15 - c : 31 - c],
                    rhs=th[:, h2 * BLK : (h2 + 1) * BLK],
                    start=False,
                    stop=(s == NSUP - 1 and o == OC - 1 and h2 == 1),
                )

        pending = None
        for s in range(NSUP):
            b = s // 2
            last_sup = s == NSUP - 1
            z = None
            for o in range(OC):
                psE = psE_pool.tile([P, SUP], F32, tag="E")
                th = tanh_pool.tile([P, SUP], F16, tag="tanh")

                def lhsT_for(kp, o=o):
                    return (
                        wet0_sb[:, 2 * kp : 2 * kp + 2, :]
                        if o == 0
                        else wetp_sb[:, 2 * kp : 2 * kp + 2, o * P : (o + 1) * P]
                    )

                # kp-outer / h2-inner: each LDWEIGHTS feeds two
                # N=512 matmuls (PSUM caps the output at one bank)
                for kp in range(KP):
                    for h2 in range(SUP // BLK):
                        nc.tensor.matmul(
                            psE[:, h2 * BLK : (h2 + 1) * BLK],
                            lhsT=lhsT_for(kp),
                            rhs=enc_sb[s][
                                :,
                                2 * kp : 2 * kp + 2,
                                h2 * BLK : (h2 + 1) * BLK,
                            ],
                            start=(kp == 0),
                            stop=(kp == KP - 1),
                            perf_mode=DR,
                        )
                nc.scalar.activation(
                    th[:],
                    psE[:],
                    AF.Tanh,
                    bias=q_sb[:, o * NB + b : o * NB + b + 1],
                    scale=1.0 / (SE * SW),
                )
                if not last_sup:
                    if o == 0:
                        z = tanh_pool.tile([P, SUP], F16, tag="z", bufs=3)
                        nc.vector.tensor_scalar_mul(z[:], th[:], vp_sb[:, 0:1])
                    else:
                        nc.vector.scalar_tensor_tensor(
                            z[:],
                            th[:],
                            vp_sb[:, o : o + 1],
                            z[:],
                            op0=mybir.AluOpType.mult,
                            op1=mybir.AluOpType.add,
                        )
                if pending is not None and o == 1:
                    emit_zmm(*pending)
                    pending = None
                if last_sup and o > 0:
                    emit_vdot(prev_th, s, o - 1)
                prev_th = th
            if not last_sup:
                pending = (z, s, b)
        emit_vdot(prev_th, NSUP - 1, OC - 1)

        # softmax on the (16, 512) layout; scores are bounded (|s| <=
        # ||v||_1) so no max-subtraction is needed in f32
        ex16 = sm_pool.tile([16, BLK], F32, tag="ex16")
        sums16 = sm_pool.tile([16, 1], F32, tag="sums16")
        nc.scalar.activation(ex16[:], psS16[:], AF.Exp, accum_out=sums16[:])
        # per-batch sums: contract the 4 j-rows of each batch on PE
        psT = psS_pool.tile([NB, 1], F32, tag="t", name="psT")
        nc.tensor.matmul(
            psT[:], lhsT=selb2_sb[:], rhs=sums16[:], start=True, stop=True
        )
        rec4 = sm_pool.tile([NB, 1], F32, tag="rec4")
        nc.vector.reciprocal(rec4[:], psT[:])
        # broadcast 1/sum back to the 16 rows
        psB = psS_pool.tile([16, 1], F32, tag="t", name="psB")
        nc.tensor.matmul(
            psB[:], lhsT=selb_sb[:], rhs=rec4[:], start=True, stop=True
        )
        probs16 = sm_pool.tile([16, BLK], F32, tag="probs16")
        nc.vector.tensor_scalar_mul(probs16[:], ex16[:], psB[:, 0:1])
        nc.sync.dma_start(
            out[:, :].rearrange("b (j t) -> (b j) t", j=4), probs16[:]
        )

    nc.compile()
    return nc


_NC = None


def _get_nc():
    global _NC
    if _NC is None:
        _NC = _build()
    return _NC


def _shard_inputs(hidden, encoder_outputs, W_attn, b_attn, v):
    hidden = np.asarray(hidden, dtype=np.float32)
    encoder_outputs = np.asarray(encoder_outputs, dtype=np.float32)
    W_attn = np.asarray(W_attn, dtype=np.float32)
    b_attn = np.asarray(b_attn, dtype=np.float32)
    v = np.asarray(v, dtype=np.float32)

    # wetp[p, k, o] = fp8(SW * W_e^T[k*128+p, o])
    wet_t = (W_attn[:, H:].T * SW).astype(NP8)  # (H, H) [h, o]
    wetp = np.ascontiguousarray(
        wet_t.reshape(KC, P, H).transpose(1, 0, 2)
    )
    wet0 = np.ascontiguousarray(
        wet_t.reshape(KC, P, H)[:, :, :P].transpose(1, 0, 2)
    )

    # q computed exactly on the host: (B, H)
    q_full = (
        hidden[0].astype(np.float64) @ W_attn[:, :H].astype(np.float64).T
        + b_attn.astype(np.float64)
    ).astype(np.float32)

    # packed constant block, f32 view (P, 128) / f16 view (P, 256)
    consts = np.zeros((P, 128), dtype=np.float32)
    c16 = consts.view(np.float16)  # (P, 256)
    consts[:, 16:20] = v.reshape(OC, P).T
    for b in range(NB):
        for j in range(NB):
            consts[NB * b + j, 20 + b] = 1.0  # selb2 (rows 0:16)
            consts[b, 24 + NB * b + j] = 1.0  # selb (rows 0:4)
    c16[:, 80 + 15] = np.float16(1.0)  # onesw: ones at f16-col 15
    vrT = v.reshape(OC, P).T.astype(np.float16)  # (P, OC)
    for o in range(OC):
        c16[:, 128 + 32 * o + 15] = vrT[:, o]  # vwin sliding windows

    # (H, B, T) so per-core slices are cheap views before the copy
    enc_hbt = np.transpose(encoder_outputs, (2, 1, 0))
    enc_all8 = (enc_hbt * SE).astype(NP8)  # one fp8 cast for all cores
    in_maps = []
    for c in range(NCORES):
        b0 = c * NB
        # enct[p, k, r] = fp8(SE * enc^T[k*128+p, r])
        enct = np.ascontiguousarray(
            enc_all8[:, b0 : b0 + NB, :]
        ).reshape(KC, P, ROWS)
        enct = np.ascontiguousarray(enct.transpose(1, 0, 2))
        cc = consts.copy()
        # qrep[p, o*NB+b] = q[b0+b, o*128+p]
        cc[:, 0:16] = (
            q_full[b0 : b0 + NB, :].T.reshape(OC, P, NB)
            .transpose(1, 0, 2).reshape(P, OC * NB)
        )
        in_maps.append(
            {
                "enct": enct,
                "wetp": wetp,
                "wet0": wet0,
                "consts": cc,
            }
        )
    return in_maps


def kernel(hidden, encoder_outputs, W_attn, b_attn, v):
    nc = _get_nc()
    in_maps = _shard_inputs(hidden, encoder_outputs, W_attn, b_attn, v)
    res = bass_utils.run_bass_kernel_spmd(
        nc, in_maps, core_ids=list(range(NCORES))
    )
    outs = [res.results[c]["out"] for c in range(NCORES)]  # each (NB, T)
    full = np.concatenate(outs, axis=0)  # (B, T)
    return full[:, None, :].astype(np.float32)  # (B, 1, T)



# revision 27
# speedup vs baseline: 1.1585x; 1.1585x over previous
"""Trainium2 Bass kernel for nn_Attn_6545530159401.

Computation (reference):
    enc  = encoder_outputs.transpose(1,0,2)            # (B,T,H)
    cat  = concat([hidden broadcast, enc], -1)         # (B,T,2H)
    en   = tanh(cat @ W_attn.T + b_attn)               # (B,T,H)
    sc   = en @ v                                      # (B,T)
    out  = softmax(sc, axis=1)[:, None, :]             # (B,1,T)

Split W_attn = [W_h | W_e] (each (H,H)):
    q[b]     = hidden[b] @ W_h.T + b_attn              # computed on HOST
    E[b,t]   = enc[b,t] @ W_e.T                        # the big matmul
    sc[b,t]  = sum_o v[o] * tanh(q[b,o] + E[b,t,o])

Sharding: data-parallel over B across 8 NeuronCores (4 batches/core),
no collectives. q is a (B, H) matvec on the inputs — done exactly on
the host and shipped inside the consts block, so the device has no
q path at all. Per-core pipeline (o-chunks on PSUM partitions so q
rides the ACT bias port):
    mains   PE: E-psum (128 o, 1024 rows) accumulated over 2 k-pairs as
            fp8e4 DoubleRow matmuls (256-deep contraction each, 2x the
            bf16 MAC rate; enc scaled x16, W_e x32 on the host, undone
            by the tanh ACTIVATE's scale=1/512); kp-outer/h2-inner so
            each LDWEIGHTS feeds two N=512 matmuls
    tanh    ACT: tanh(E/512 + q[b, o-chunk]) via per-partition bias
    z       GPSIMD seeds zB = v_0 * tanh_0, then DVE chains the three
            remaining o-chunks with fused scalar_tensor_tensor
            (balances DVE against the ACT 4us/superblock cadence)
    scores  PE: sliding-window ones lhsT contracts z's 128 partitions
            into row 2s+h2 of one persistent (16, 512) psum tile
            (last superblock v-dots straight from tanh so the tail never
            waits on the DVE chain); all emissions pipelined one
            iteration behind so the in-order PE never waits on ACT/DVE
    softmax ACT exp with fused accum sum on (16, 512), per-batch
            sum/broadcast via tiny PE matmuls, no max-subtraction
            (scores are bounded by ||v||_1)
Tiny-N warmup matmuls run inside the head-DMA shadow so the PE clock
gate (HAM) releases before the first real matmuls; head DMAs are split
across the two HWDGE rings and ordered by what the PE needs first.
"""

import numpy as np
import ml_dtypes
from contextlib import ExitStack

import concourse.bass as bass
import concourse.tile as tile
from concourse import bacc, mybir
import concourse.bass_utils as bass_utils

T, B, H = 2048, 32, 512
NCORES = 8
NB = B // NCORES        # 4 local batches per core
ROWS = NB * T           # 8192 rows per core
P = 128
KC = H // P             # 4 contraction chunks
KP = KC // 2            # 2 DoubleRow k-pairs (256-deep each)
OC = H // P             # 4 output chunks
SUP = 1024              # columns per E-psum tile (2 PSUM banks)
NSUP = ROWS // SUP      # 8
BLK = 512               # matmul moving-dim limit (one PSUM bank)
SE = 16.0               # host-side fp8 scale on enc
SW = 32.0               # host-side fp8 scale on W_e
F32 = mybir.dt.float32
F32R = mybir.dt.float32r
F16 = mybir.dt.float16
F8 = mybir.dt.float8e4
NP8 = ml_dtypes.float8_e4m3
DR = mybir.MatmulPerfMode.DoubleRow
AF = mybir.ActivationFunctionType


def _build():
    nc = bacc.Bacc(
        "TRN2", target_bir_lowering=False, debug=False, num_devices=NCORES
    )
    # enct is packed k-chunk-major: enct[p, k, r] = fp8(SE * enc^T[k*128+p, r])
    enct = nc.declare_dram_parameter("enct", [P, KC, ROWS], F8, isOutput=False)
    # wetp packs the 4 k-chunks side by side: [p, k, o] = fp8(SW * W_e^T[k*128+p, o])
    wetp = nc.declare_dram_parameter("wetp", [P, KC, H], F8, isOutput=False)
    # o=0 slice of wetp: wet0[p, k, o] = fp8(SW * W_e^T[k*128+p, o]) (o<128)
    wet0 = nc.declare_dram_parameter("wet0", [P, KC, P], F8, isOutput=False)
    # all small constants in one (P, 128) f32 block (f16 regions bitcast):
    #   f32 cols  0:16  qrep[p, o*NB+b] = q[b0+b, o*128+p]  (host-computed)
    #   f32 cols 16:20  vp[p, o] = v[o*128+p]
    #   f32 cols 20:24  selb2 (rows 0:16)
    #   f32 cols 24:40  selb  (rows 0:4)
    #   f32 cols 40:56  onesw as f16[32]: ones at f16-col 15 (sliding lhsT)
    #   f32 cols 64:128 vwin: 4 chunks of 32 f16 cols, chunk o has
    #                   v[o*128+p] at f16-col 128+32*o+15 (sliding lhsT)
    consts = nc.declare_dram_parameter("consts", [P, 128], F32, isOutput=False)
    out = nc.declare_dram_parameter("out", [NB, T], F32, isOutput=True)

    with tile.TileContext(nc) as tc, ExitStack() as ctx:
        const_pool = ctx.enter_context(tc.tile_pool(name="const", bufs=1))
        enc_pool = ctx.enter_context(tc.tile_pool(name="enc", bufs=1))
        tanh_pool = ctx.enter_context(tc.tile_pool(name="tanh", bufs=6))
        sm_pool = ctx.enter_context(tc.tile_pool(name="sm", bufs=1))
        psE_pool = ctx.enter_context(tc.tile_pool(name="psE", bufs=3, space="PSUM"))
        psS_pool = ctx.enter_context(tc.tile_pool(name="psS", bufs=1, space="PSUM"))

        # o=0 weight slice lands first (small), in parallel (scalar ring)
        # with the k-chunks of the first enc superblock (sync ring), so the
        # first main matmuls can start as early as possible.
        wet0_sb = const_pool.tile([P, KC, P], F8, tag="wet0")
        nc.scalar.dma_start(wet0_sb[:], wet0[:, :, :])

        def load_sup(s, engine=None):
            e = enc_pool.tile([P, KC, SUP], F8, tag=f"e{s}", name=f"e{s}")
            (engine or nc.sync).dma_start(
                e[:], enct[:, :, s * SUP : (s + 1) * SUP]
            )
            return e

        # whole enc shard stays SBUF-resident (4 * 4KB/partition = 16KB);
        # one 512KB DMA per superblock. The first superblock is split per
        # k-chunk across BOTH rings (kp0 on sync, kp1 on scalar) so both
        # DoubleRow k-pairs land as early as possible.
        enc_sb = [None] * NSUP
        e0 = enc_pool.tile([P, KC, SUP], F8, tag="e0", name="e0")
        for k in range(KC):
            nc.sync.dma_start(e0[:, k, :], enct[:, k, 0:SUP])
        enc_sb[0] = e0

        c_sb = const_pool.tile([P, 128], F32, tag="consts")
        nc.scalar.dma_start(c_sb[:], consts[:, :])
        c16 = c_sb[:].bitcast(F16)  # (P, 256) f16 view
        q_sb = c_sb[:, 0:16]
        vp_sb = c_sb[:, 16:20]
        selb2_sb = c_sb[0:16, 20:24]
        selb_sb = c_sb[0:4, 24:40]
        onesw_sb = c16[:, 80:111]
        vwin_sb = [c16[:, 128 + 32 * o : 159 + 32 * o] for o in range(OC)]

        wetp_sb = const_pool.tile([P, KC, H], F8, tag="wetp")
        nc.scalar.dma_start(wetp_sb[:], wetp[:, :, :])

        # late superblocks ride the scalar ring (it is idle after the
        # constant loads), halving the enc DMA tail on the sync ring
        for s in range(1, NSUP):
            enc_sb[s] = load_sup(
                s, engine=nc.scalar if s >= NSUP // 2 else nc.sync
            )

        # PE warmup: tiny N=64 matmuls in the head-DMA shadow keep the PE
        # queue primed; the first (DMA-paced) real superblocks then warm
        # the HAM clock gate.
        warm = const_pool.tile([P, 64], F16, tag="warm")
        nc.gpsimd.memset(warm[:], 0.0)
        psW = psS_pool.tile([P, 64], F32, tag="t", name="psW")
        for _ in range(8):
            nc.tensor.matmul(
                psW[0:64, :], lhsT=warm[:, 0:64], rhs=warm[:],
                start=True, stop=True,
            )

        # single persistent score accumulator: row 2s+h2 = scores of
        # (batch s//2, t-slice (s%2)*1024 + h2*512)
        psS16 = psS_pool.tile([16, BLK], F32, tag="s16", name="psS16")

        # z(s) = sum_o v_o * tanh(E_o + q_o) is built on DVE (fused
        # scalar_tensor_tensor), then one small matmul per 512-block
        # contracts the 128 partitions into psS. The z-matmul for s is
        # emitted during s+1's main matmuls so the in-order PE never
        # waits on the DVE chain it just scheduled.
        def emit_zmm(z, s, b):
            for h2 in range(SUP // BLK):
                c = 2 * s + h2
                nc.tensor.matmul(
                    psS16[:],
                    lhsT=onesw_sb[:, 15 - c : 31 - c],
                    rhs=z[:, h2 * BLK : (h2 + 1) * BLK],
                    start=(s == 0 and h2 == 0),
                    stop=(s == NSUP - 1 and h2 == 1),
                )

        def emit_vdot(th, s, o):
            # direct PE v-dot (used for the last superblock so the tail
            # does not wait on the DVE z-chain)
            for h2 in range(SUP // BLK):
                c = 2 * s + h2
                nc.tensor.matmul(
                    psS16[:],
                    lhsT=vwin_sb[o][:, 15 - c : 31 - c],
                    rhs=th[:, h2 * BLK : (h2 + 1) * BLK],
                    start=False,
                    stop=(s == NSUP - 1 and o == OC - 1 and h2 == 1),
                )

        pending = None
        for s in range(NSUP):
            b = s // 2
            last_sup = s == NSUP - 1
            z = None
            for o in range(OC):
                psE = psE_pool.tile([P, SUP], F32, tag="E")
                th = tanh_pool.tile([P, SUP], F16, tag="tanh")

                def lhsT_for(kp, o=o):
                    return (
                        wet0_sb[:, 2 * kp : 2 * kp + 2, :]
                        if o == 0
                        else wetp_sb[:, 2 * kp : 2 * kp + 2, o * P : (o + 1) * P]
                    )

                # kp-outer / h2-inner: each LDWEIGHTS feeds two
                # N=512 matmuls (PSUM caps the output at one bank)
                for kp in range(KP):
                    for h2 in range(SUP // BLK):
                        nc.tensor.matmul(
                            psE[:, h2 * BLK : (h2 + 1) * BLK],
                            lhsT=lhsT_for(kp),
                            rhs=enc_sb[s][
                                :,
                                2 * kp : 2 * kp + 2,
                                h2 * BLK : (h2 + 1) * BLK,
                            ],
                            start=(kp == 0),
                            stop=(kp == KP - 1),
                            perf_mode=DR,
                        )
                nc.scalar.activation(
                    th[:],
                    psE[:],
                    AF.Tanh,
                    bias=q_sb[:, o * NB + b : o * NB + b + 1],
                    scale=1.0 / (SE * SW),
                )
                if not last_sup:
                    if o == 0:
                        z = tanh_pool.tile([P, SUP], F16, tag="z", bufs=3)
                        nc.vector.tensor_scalar_mul(z[:], th[:], vp_sb[:, 0:1])
                    else:
                        nc.vector.scalar_tensor_tensor(
                            z[:],
                            th[:],
                            vp_sb[:, o : o + 1],
                            z[:],
                            op0=mybir.AluOpType.mult,
                            op1=mybir.AluOpType.add,
                        )
                if pending is not None and o == 1:
                    emit_zmm(*pending)
                    pending = None
                if last_sup and o > 0:
                    emit_vdot(prev_th, s, o - 1)
                prev_th = th
            if not last_sup:
                pending = (z, s, b)
        emit_vdot(prev_th, NSUP - 1, OC - 1)

        # softmax on the (16, 512) layout; scores are bounded (|s| <=
        # ||v||_1) so no max-subtraction is needed in f32
        ex16 = sm_pool.tile([16, BLK], F32, tag="ex16")
        sums16 = sm_pool.tile([16, 1], F32, tag="sums16")
        nc.scalar.activation(ex16[:], psS16[:], AF.Exp, accum_out=sums16[:])
        # per-batch sums: contract the 4 j-rows of each batch on PE
        psT = psS_pool.tile([NB, 1], F32, tag="t", name="psT")
        nc.tensor.matmul(
            psT[:], lhsT=selb2_sb[:], rhs=sums16[:], start=True, stop=True
        )
        rec4 = sm_pool.tile([NB, 1], F32, tag="rec4")
        nc.vector.reciprocal(rec4[:], psT[:])
        # broadcast 1/sum back to the 16 rows
        psB = psS_pool.tile([16, 1], F32, tag="t", name="psB")
        nc.tensor.matmul(
            psB[:], lhsT=selb_sb[:], rhs=rec4[:], start=True, stop=True
        )
        probs16 = sm_pool.tile([16, BLK], F32, tag="probs16")
        nc.vector.tensor_scalar_mul(probs16[:], ex16[:], psB[:, 0:1])
        nc.sync.dma_start(
            out[:, :].rearrange("b (j t) -> (b j) t", j=4), probs16[:]
        )

    nc.compile()
    return nc


_NC = None


def _get_nc():
    global _NC
    if _NC is None:
        _NC = _build()
    return _NC


def _shard_inputs(hidden, encoder_outputs, W_attn, b_attn, v):
    hidden = np.asarray(hidden, dtype=np.float32)
    encoder_outputs = np.asarray(encoder_outputs, dtype=np.float32)
    W_attn = np.asarray(W_attn, dtype=np.float32)
    b_attn = np.asarray(b_attn, dtype=np.float32)
    v = np.asarray(v, dtype=np.float32)

    # wetp[p, k, o] = fp8(SW * W_e^T[k*128+p, o])
    wet_t = (W_attn[:, H:].T * SW).astype(NP8)  # (H, H) [h, o]
    wetp = np.ascontiguousarray(
        wet_t.reshape(KC, P, H).transpose(1, 0, 2)
    )
    wet0 = np.ascontiguousarray(
        wet_t.reshape(KC, P, H)[:, :, :P].transpose(1, 0, 2)
    )

    # q computed exactly on the host: (B, H)
    q_full = (
        hidden[0].astype(np.float64) @ W_attn[:, :H].astype(np.float64).T
        + b_attn.astype(np.float64)
    ).astype(np.float32)

    # packed constant block, f32 view (P, 128) / f16 view (P, 256)
    consts = np.zeros((P, 128), dtype=np.float32)
    c16 = consts.view(np.float16)  # (P, 256)
    consts[:, 16:20] = v.reshape(OC, P).T
    for b in range(NB):
        for j in range(NB):
            consts[NB * b + j, 20 + b] = 1.0  # selb2 (rows 0:16)
            consts[b, 24 + NB * b + j] = 1.0  # selb (rows 0:4)
    c16[:, 80 + 15] = np.float16(1.0)  # onesw: ones at f16-col 15
    vrT = v.reshape(OC, P).T.astype(np.float16)  # (P, OC)
    for o in range(OC):
        c16[:, 128 + 32 * o + 15] = vrT[:, o]  # vwin sliding windows

    # (H, B, T) so per-core slices are cheap views before the copy
    enc_hbt = np.transpose(encoder_outputs, (2, 1, 0))
    enc_all8 = (enc_hbt * SE).astype(NP8)  # one fp8 cast for all cores
    in_maps = []
    for c in range(NCORES):
        b0 = c * NB
        # enct[p, k, r] = fp8(SE * enc^T[k*128+p, r])
        enct = np.ascontiguousarray(
            enc_all8[:, b0 : b0 + NB, :]
        ).reshape(KC, P, ROWS)
        enct = np.ascontiguousarray(enct.transpose(1, 0, 2))
        cc = consts.copy()
        # qrep[p, o*NB+b] = q[b0+b, o*128+p]
        cc[:, 0:16] = (
            q_full[b0 : b0 + NB, :].T.reshape(OC, P, NB)
            .transpose(1, 0, 2).reshape(P, OC * NB)
        )
        in_maps.append(
            {
                "enct": enct,
                "wetp": wetp,
                "wet0": wet0,
                "consts": cc,
            }
        )
    return in_maps


def kernel(hidden, encoder_outputs, W_attn, b_attn, v):
    nc = _get_nc()
    in_maps = _shard_inputs(hidden, encoder_outputs, W_attn, b_attn, v)
    res = bass_utils.run_bass_kernel_spmd(
        nc, in_maps, core_ids=list(range(NCORES))
    )
    outs = [res.results[c]["out"] for c in range(NCORES)]  # each (NB, T)
    full = np.concatenate(outs, axis=0)  # (B, T)
    return full[:, None, :].astype(np.float32)  # (B, 1, T)



# revision 59
# speedup vs baseline: 1.2473x; 1.0766x over previous
"""Trainium2 Bass kernel for nn_Attn_6545530159401.

Computation (reference):
    enc  = encoder_outputs.transpose(1,0,2)            # (B,T,H)
    cat  = concat([hidden broadcast, enc], -1)         # (B,T,2H)
    en   = tanh(cat @ W_attn.T + b_attn)               # (B,T,H)
    sc   = en @ v                                      # (B,T)
    out  = softmax(sc, axis=1)[:, None, :]             # (B,1,T)

Split W_attn = [W_h | W_e] (each (H,H)):
    q[b]     = hidden[b] @ W_h.T + b_attn              # computed on HOST
    E[b,t]   = enc[b,t] @ W_e.T                        # the big matmul
    sc[b,t]  = sum_o v[o] * tanh(q[b,o] + E[b,t,o])

Sharding: data-parallel over B across 8 NeuronCores (4 batches/core),
no collectives. q is a (B, H) matvec on the inputs — done exactly on
the host and shipped inside the consts block, so the device has no
q path at all. Per-core pipeline (o-chunks on PSUM partitions so q
rides the ACT bias port):
    mains   PE: E-psum (128 o, 1024 rows) accumulated over 2 k-pairs as
            fp8e4 DoubleRow matmuls (256-deep contraction each, 2x the
            bf16 MAC rate; enc scaled x16, W_e x32 on the host, undone
            by the tanh ACTIVATE's scale=1/512); kp-outer/h2-inner so
            each LDWEIGHTS feeds two N=512 matmuls
    tanh    ACT: tanh(E/512 + q[b, o-chunk]) via per-partition bias
    z       GPSIMD seeds zB = v_0 * tanh_0, then DVE chains the three
            remaining o-chunks with fused scalar_tensor_tensor
            (balances DVE against the ACT 4us/superblock cadence)
    scores  PE: sliding-window ones lhsT contracts z's 128 partitions
            into row 2s+h2 of one persistent (16, 512) psum tile
            (last superblock v-dots straight from tanh so the tail never
            waits on the DVE chain); all emissions pipelined one
            iteration behind so the in-order PE never waits on ACT/DVE
    softmax ACT exp with fused accum sum on (16, 512), per-batch
            sum/broadcast via tiny PE matmuls, no max-subtraction
            (scores are bounded by ||v||_1)
Tiny-N warmup matmuls run inside the head-DMA shadow so the PE clock
gate (HAM) releases before the first real matmuls; head DMAs are split
across the two HWDGE rings and ordered by what the PE needs first.
"""

import numpy as np
import ml_dtypes
from contextlib import ExitStack

import concourse.bass as bass
import concourse.tile as tile
from concourse import bacc, mybir
import concourse.bass_utils as bass_utils

T, B, H = 2048, 32, 512
NCORES = 8
NB = B // NCORES        # 4 local batches per core
ROWS = NB * T           # 8192 rows per core
P = 128
KC = H // P             # 4 contraction chunks
KP = KC // 2            # 2 DoubleRow k-pairs (256-deep each)
OC = H // P             # 4 output chunks
SUP = 1024              # columns per E-psum tile (2 PSUM banks)
NSUP = ROWS // SUP      # 8
BLK = 512               # matmul moving-dim limit (one PSUM bank)
SE = 16.0               # host-side fp8 scale on enc
SW = 32.0               # host-side fp8 scale on W_e
F32 = mybir.dt.float32
F32R = mybir.dt.float32r
F16 = mybir.dt.float16
F8 = mybir.dt.float8e4
NP8 = ml_dtypes.float8_e4m3
DR = mybir.MatmulPerfMode.DoubleRow
AF = mybir.ActivationFunctionType


def _build():
    nc = bacc.Bacc(
        "TRN2", target_bir_lowering=False, debug=False, num_devices=NCORES
    )
    # enct is packed k-chunk-major: enct[p, k, r] = fp8(SE * enc^T[k*128+p, r])
    enct = nc.declare_dram_parameter("enct", [P, KC, ROWS], F8, isOutput=False)
    # wetp packs the 4 k-chunks side by side for o >= 128 only (the o=0
    # slice ships inside consts): [p, k, o-128] = fp8(SW * W_e^T[k*128+p, o])
    wetp = nc.declare_dram_parameter("wetp", [P, KC, H - P], F8, isOutput=False)
    # all small constants in one (P, 256) f32 block (other dtypes bitcast)
    # so ONE head DMA delivers everything the first matmul group needs:
    #   f32 cols  0:16  qrep[p, o*NB+b] = q[b0+b, o*128+p]  (host-computed)
    #   f32 cols 16:20  vp[p, o] = v[o*128+p]
    #   f32 cols 20:36  m16 (rows 0:16): block-diagonal per-batch ones
    #   f32 cols 40:56  onesw as f16[32]: ones at f16-col 15 (sliding lhsT)
    #   f32 cols 64:128 vwin: 4 chunks of 32 f16 cols, chunk o has
    #                   v[o*128+p] at f16-col 128+32*o+15 (sliding lhsT)
    #   f32 cols 128:256  wet0 as fp8 [k*128+o]: the o=0 slice of wetp,
    #                   wet0[p, k, o] = fp8(SW * W_e^T[k*128+p, o]) (o<128)
    consts = nc.declare_dram_parameter("consts", [P, 256], F32, isOutput=False)
    out = nc.declare_dram_parameter("out", [NB, T], F32, isOutput=True)

    with tile.TileContext(nc) as tc, ExitStack() as ctx:
        const_pool = ctx.enter_context(tc.tile_pool(name="const", bufs=1))
        enc_pool = ctx.enter_context(tc.tile_pool(name="enc", bufs=1))
        tanh_pool = ctx.enter_context(tc.tile_pool(name="tanh", bufs=6))
        sm_pool = ctx.enter_context(tc.tile_pool(name="sm", bufs=1))
        psE_pool = ctx.enter_context(tc.tile_pool(name="psE", bufs=3, space="PSUM"))
        psS_pool = ctx.enter_context(tc.tile_pool(name="psS", bufs=1, space="PSUM"))

        def load_sup(s, engine=None):
            e = enc_pool.tile([P, KC, SUP], F8, tag=f"e{s}", name=f"e{s}")
            (engine or nc.sync).dma_start(
                e[:], enct[:, :, s * SUP : (s + 1) * SUP]
            )
            return e

        # whole enc shard stays SBUF-resident (4 * 4KB/partition = 16KB);
        # one 512KB DMA per superblock. The first superblock is split per
        # k-chunk across BOTH rings (kp0 on sync, kp1 on scalar behind
        # the small wet0) so both DoubleRow k-pairs land in ~2 DMA slots
        # and the (s0, o0) group closes ~4us earlier. The tiny consts
        # block leads the sync ring so the q bias beats the first tanh.
        enc_sb = [None] * NSUP
        # the consts+wet0 block leads the sync ring ahead of e0's kp0;
        # each e0 k-PAIR moves as ONE strided DMA (ring completions
        # pipeline ~1.2us apart, so one 256KB transfer beats two 128KB
        # slots), kp0 on sync and kp1 leading the scalar ring
        c_sb = const_pool.tile([P, 256], F32, tag="consts")
        nc.sync.dma_start(c_sb[:], consts[:, :])
        e0 = enc_pool.tile([P, KC, SUP], F8, tag="e0", name="e0")
        nc.scalar.dma_start(e0[:, 2:KC, :], enct[:, 2:KC, 0:SUP])
        nc.sync.dma_start(e0[:, 0:2, :], enct[:, 0:2, 0:SUP])
        enc_sb[0] = e0
        c16 = c_sb[:].bitcast(F16)  # (P, 512) f16 view
        q_sb = c_sb[:, 0:16]
        vp_sb = c_sb[:, 16:20]
        m16_sb = c_sb[0:16, 20:36]
        onesw_sb = c16[:, 80:111]
        vwin_sb = [c16[:, 128 + 32 * o : 159 + 32 * o] for o in range(OC)]
        wet0_sb = (
            c_sb[:, 128:256].bitcast(F8).rearrange("p (k o) -> p k o", k=KC)
        )

        wetp_sb = const_pool.tile([P, KC, H - P], F8, tag="wetp")
        nc.scalar.dma_start(wetp_sb[:], wetp[:, :, :])

        # late superblocks ride the scalar ring (it is idle after the
        # constant loads), halving the enc DMA tail on the sync ring
        for s in range(1, NSUP):
            enc_sb[s] = load_sup(
                s, engine=nc.scalar if s >= NSUP // 2 else nc.sync
            )

        # PE warmup: 8 N=512 matmuls on a zeroed tile fill the ~3us
        # head-DMA shadow with enough sustained activity to release the
        # HAM clock gate (2.4 GHz) before the first real matmuls.
        warm = const_pool.tile([P, BLK], F16, tag="warm")
        nc.gpsimd.memset(warm[:], 0.0)
        psW = psS_pool.tile([P, BLK], F32, tag="t", name="psW")
        for _ in range(8):
            nc.tensor.matmul(
                psW[:], lhsT=warm[:, 0:P], rhs=warm[:],
                start=True, stop=True,
            )

        # single persistent score accumulator: row 2s+h2 = scores of
        # (batch s//2, t-slice (s%2)*1024 + h2*512)
        psS16 = psS_pool.tile([16, BLK], F32, tag="s16", name="psS16")

        # z(s) = sum_o v_o * tanh(E_o + q_o) is built on DVE (fused
        # scalar_tensor_tensor), then one small matmul per 512-block
        # contracts the 128 partitions into psS. The z-matmul for s is
        # emitted during s+1's main matmuls so the in-order PE never
        # waits on the DVE chain it just scheduled.
        def emit_zmm(z, s, b):
            for h2 in range(SUP // BLK):
                c = 2 * s + h2
                nc.tensor.matmul(
                    psS16[:],
                    lhsT=onesw_sb[:, 15 - c : 31 - c],
                    rhs=z[:, h2 * BLK : (h2 + 1) * BLK],
                    start=(s == 0 and h2 == 0),
                    stop=(s == NSUP - 1 and h2 == 1),
                )

        def emit_vdot(th, s, o):
            # direct PE v-dot (used for the last superblock so the tail
            # does not wait on the DVE z-chain)
            for h2 in range(SUP // BLK):
                c = 2 * s + h2
                nc.tensor.matmul(
                    psS16[:],
                    lhsT=vwin_sb[o][:, 15 - c : 31 - c],
                    rhs=th[:, h2 * BLK : (h2 + 1) * BLK],
                    start=False,
                    stop=(s == NSUP - 1 and o == OC - 1 and h2 == 1),
                )

        # Tile schedule: each superblock's o0 is hoisted one slot early
        # ((s,3) runs after (s+1,0)) so the ACT engine rolls straight
        # from sup s into sup s+1 without waiting for s+1's first mains
        # at the boundary — PE produces E(s+1,o0) one ACT-slot ahead.
        sched = [(0, 0), (0, 1), (0, 2)]
        for s in range(NSUP - 1):
            sched += [(s + 1, 0), (s, 3), (s + 1, 1), (s + 1, 2)]
        sched += [(NSUP - 1, 3)]

        pending = None
        zt = {}          # live z tile per superblock
        th7 = {}         # last-superblock tanh tiles for the PE v-dot
        for s, o in sched:
            b = s // 2
            last_sup = s == NSUP - 1
            psE = psE_pool.tile([P, SUP], F32, tag="E")
            th = tanh_pool.tile([P, SUP], F16, tag="tanh")

            def lhsT_for(kp, o=o):
                return (
                    wet0_sb[:, 2 * kp : 2 * kp + 2, :]
                    if o == 0
                    else wetp_sb[:, 2 * kp : 2 * kp + 2, (o - 1) * P : o * P]
                )

            # kp-outer / h2-inner: each LDWEIGHTS feeds two
            # N=512 matmuls (PSUM caps the output at one bank)
            for kp in range(KP):
                for h2 in range(SUP // BLK):
                    nc.tensor.matmul(
                        psE[:, h2 * BLK : (h2 + 1) * BLK],
                        lhsT=lhsT_for(kp),
                        rhs=enc_sb[s][
                            :,
                            2 * kp : 2 * kp + 2,
                            h2 * BLK : (h2 + 1) * BLK,
                        ],
                        start=(kp == 0),
                        stop=(kp == KP - 1),
                        perf_mode=DR,
                    )
            nc.scalar.activation(
                th[:],
                psE[:],
                AF.Tanh,
                bias=q_sb[:, o * NB + b : o * NB + b + 1],
                scale=1.0 / (SE * SW),
            )
            if not last_sup:
                if o == 0:
                    zt[s] = tanh_pool.tile(
                        [P, SUP], F16, tag="z", bufs=3, name=f"z{s}"
                    )
                    nc.vector.tensor_scalar_mul(zt[s][:], th[:], vp_sb[:, 0:1])
                else:
                    nc.vector.scalar_tensor_tensor(
                        zt[s][:],
                        th[:],
                        vp_sb[:, o : o + 1],
                        zt[s][:],
                        op0=mybir.AluOpType.mult,
                        op1=mybir.AluOpType.add,
                    )
                if o == 3:
                    pending = (zt.pop(s), s, b)
            else:
                # defer ALL last-superblock v-dots until after its final
                # mains: ACT(7,o3) then gates only the 2-matmul stop pair
                # before the softmax exp can fire
                th7[o] = th
            if pending is not None and o == 1:
                emit_zmm(*pending)
                pending = None
        for o in range(OC):
            emit_vdot(th7.pop(o), NSUP - 1, o)

        # softmax on the (16, 512) layout; scores are bounded (|s| <=
        # ||v||_1) so no max-subtraction is needed in f32
        ex16 = sm_pool.tile([16, BLK], F32, tag="ex16")
        sums16 = sm_pool.tile([16, 1], F32, tag="sums16")
        nc.scalar.activation(ex16[:], psS16[:], AF.Exp, accum_out=sums16[:])
        # one block-diagonal (16,16) matmul fuses contract-per-batch and
        # broadcast-back-to-rows; then a single reciprocal feeds the mult
        psB = psS_pool.tile([16, 1], F32, tag="t", name="psB")
        nc.tensor.matmul(
            psB[:], lhsT=m16_sb[:], rhs=sums16[:], start=True, stop=True
        )
        rec16 = sm_pool.tile([16, 1], F32, tag="rec16")
        nc.vector.reciprocal(rec16[:], psB[:])
        probs16 = sm_pool.tile([16, BLK], F32, tag="probs16")
        nc.vector.tensor_scalar_mul(probs16[:], ex16[:], rec16[:, 0:1])
        nc.sync.dma_start(
            out[:, :].rearrange("b (j t) -> (b j) t", j=4), probs16[:]
        )

    nc.compile()
    return nc


_NC = None


def _get_nc():
    global _NC
    if _NC is None:
        _NC = _build()
    return _NC


def _shard_inputs(hidden, encoder_outputs, W_attn, b_attn, v):
    hidden = np.asarray(hidden, dtype=np.float32)
    encoder_outputs = np.asarray(encoder_outputs, dtype=np.float32)
    W_attn = np.asarray(W_attn, dtype=np.float32)
    b_attn = np.asarray(b_attn, dtype=np.float32)
    v = np.asarray(v, dtype=np.float32)

    # wetp[p, k, o-128] = fp8(SW * W_e^T[k*128+p, o]) for o >= 128
    wet_t = (W_attn[:, H:].T * SW).astype(NP8)  # (H, H) [h, o]
    wetp = np.ascontiguousarray(
        wet_t.reshape(KC, P, H)[:, :, P:].transpose(1, 0, 2)
    )
    wet0 = np.ascontiguousarray(
        wet_t.reshape(KC, P, H)[:, :, :P].transpose(1, 0, 2)
    )

    # q computed exactly on the host: (B, H)
    q_full = (
        hidden[0].astype(np.float64) @ W_attn[:, :H].astype(np.float64).T
        + b_attn.astype(np.float64)
    ).astype(np.float32)

    # packed constant block, f32 view (P, 256) / f16 view (P, 512)
    consts = np.zeros((P, 256), dtype=np.float32)
    c16 = consts.view(np.float16)  # (P, 512)
    consts[:, 16:20] = v.reshape(OC, P).T
    for i in range(16):
        for j in range(16):
            if i // NB == j // NB:
                consts[i, 20 + j] = 1.0  # m16: block-diagonal per batch
    c16[:, 80 + 15] = np.float16(1.0)  # onesw: ones at f16-col 15
    vrT = v.reshape(OC, P).T.astype(np.float16)  # (P, OC)
    for o in range(OC):
        c16[:, 128 + 32 * o + 15] = vrT[:, o]  # vwin sliding windows
    # wet0 (fp8, (P, KC*P) bytes) rides in f32 cols 128:256
    consts[:, 128:256] = np.ascontiguousarray(
        wet0.reshape(P, KC * P)
    ).view(np.float32)

    # (H, B, T) so per-core slices are cheap views before the copy
    enc_hbt = np.transpose(encoder_outputs, (2, 1, 0))
    enc_all8 = (enc_hbt * SE).astype(NP8)  # one fp8 cast for all cores
    in_maps = []
    for c in range(NCORES):
        b0 = c * NB
        # enct[p, k, r] = fp8(SE * enc^T[k*128+p, r])
        enct = np.ascontiguousarray(
            enc_all8[:, b0 : b0 + NB, :]
        ).reshape(KC, P, ROWS)
        enct = np.ascontiguousarray(enct.transpose(1, 0, 2))
        cc = consts.copy()
        # qrep[p, o*NB+b] = q[b0+b, o*128+p]
        cc[:, 0:16] = (
            q_full[b0 : b0 + NB, :].T.reshape(OC, P, NB)
            .transpose(1, 0, 2).reshape(P, OC * NB)
        )
        in_maps.append(
            {
                "enct": enct,
                "wetp": wetp,
                "consts": cc,
            }
        )
    return in_maps


def kernel(hidden, encoder_outputs, W_attn, b_attn, v):
    nc = _get_nc()
    in_maps = _shard_inputs(hidden, encoder_outputs, W_attn, b_attn, v)
    res = bass_utils.run_bass_kernel_spmd(
        nc, in_maps, core_ids=list(range(NCORES))
    )
    outs = [res.results[c]["out"] for c in range(NCORES)]  # each (NB, T)
    full = np.concatenate(outs, axis=0)  # (B, T)
    return full[:, None, :].astype(np.float32)  # (B, 1, T)



# revision 60
# speedup vs baseline: 1.2769x; 1.0237x over previous
"""Trainium2 Bass kernel for nn_Attn_6545530159401.

Computation (reference):
    enc  = encoder_outputs.transpose(1,0,2)            # (B,T,H)
    cat  = concat([hidden broadcast, enc], -1)         # (B,T,2H)
    en   = tanh(cat @ W_attn.T + b_attn)               # (B,T,H)
    sc   = en @ v                                      # (B,T)
    out  = softmax(sc, axis=1)[:, None, :]             # (B,1,T)

Split W_attn = [W_h | W_e] (each (H,H)):
    q[b]     = hidden[b] @ W_h.T + b_attn              # computed on HOST
    E[b,t]   = enc[b,t] @ W_e.T                        # the big matmul
    sc[b,t]  = sum_o v[o] * tanh(q[b,o] + E[b,t,o])

Sharding: data-parallel over B across 8 NeuronCores (4 batches/core),
no collectives. q is a (B, H) matvec on the inputs — done exactly on
the host and shipped inside the consts block, so the device has no
q path at all. Per-core pipeline (o-chunks on PSUM partitions so q
rides the ACT bias port):
    mains   PE: E-psum (128 o, 1024 rows) accumulated over 2 k-pairs as
            fp8e4 DoubleRow matmuls (256-deep contraction each, 2x the
            bf16 MAC rate; enc scaled x16, W_e x32 on the host, undone
            by the tanh ACTIVATE's scale=1/512); kp-outer/h2-inner so
            each LDWEIGHTS feeds two N=512 matmuls
    tanh    ACT: tanh(E/512 + q[b, o-chunk]) via per-partition bias
    z       GPSIMD seeds zB = v_0 * tanh_0, then DVE chains the three
            remaining o-chunks with fused scalar_tensor_tensor
            (balances DVE against the ACT 4us/superblock cadence)
    scores  PE: sliding-window ones lhsT contracts z's 128 partitions
            into row 2s+h2 of one persistent (16, 512) psum tile
            (last superblock v-dots straight from tanh so the tail never
            waits on the DVE chain); all emissions pipelined one
            iteration behind so the in-order PE never waits on ACT/DVE
    softmax ACT exp with fused accum sum on (16, 512), per-batch
            sum/broadcast via tiny PE matmuls, no max-subtraction
            (scores are bounded by ||v||_1)
Tiny-N warmup matmuls run inside the head-DMA shadow so the PE clock
gate (HAM) releases before the first real matmuls; head DMAs are split
across the two HWDGE rings and ordered by what the PE needs first.
"""

import numpy as np
import ml_dtypes
from contextlib import ExitStack

import concourse.bass as bass
import concourse.tile as tile
from concourse import bacc, mybir
import concourse.bass_utils as bass_utils

T, B, H = 2048, 32, 512
NCORES = 8
NB = B // NCORES        # 4 local batches per core
ROWS = NB * T           # 8192 rows per core
P = 128
KC = H // P             # 4 contraction chunks
KP = KC // 2            # 2 DoubleRow k-pairs (256-deep each)
OC = H // P             # 4 output chunks
SUP = 1024              # columns per E-psum tile (2 PSUM banks)
NSUP = ROWS // SUP      # 8
BLK = 512               # matmul moving-dim limit (one PSUM bank)
SE = 16.0               # host-side fp8 scale on enc
SW = 32.0               # host-side fp8 scale on W_e
F32 = mybir.dt.float32
F32R = mybir.dt.float32r
F16 = mybir.dt.float16
F8 = mybir.dt.float8e4
NP8 = ml_dtypes.float8_e4m3
DR = mybir.MatmulPerfMode.DoubleRow
AF = mybir.ActivationFunctionType


def _build():
    nc = bacc.Bacc(
        "TRN2", target_bir_lowering=False, debug=False, num_devices=NCORES
    )
    # enct is packed k-chunk-major: enct[p, k, r] = fp8(SE * enc^T[k*128+p, r])
    enct = nc.declare_dram_parameter("enct", [P, KC, ROWS], F8, isOutput=False)
    # wetp packs the 4 k-chunks side by side for o >= 128 only (the o=0
    # slice ships inside consts): [p, k, o-128] = fp8(SW * W_e^T[k*128+p, o])
    wetp = nc.declare_dram_parameter("wetp", [P, KC, H - P], F8, isOutput=False)
    # all small constants in one (P, 256) f32 block (other dtypes bitcast)
    # so ONE head DMA delivers everything the first matmul group needs:
    #   f32 cols  0:16  qrep[p, o*NB+b] = q[b0+b, o*128+p]  (host-computed)
    #   f32 cols 16:20  vp[p, o] = v[o*128+p]
    #   f32 cols 20:36  m16 (rows 0:16): block-diagonal per-batch ones
    #   f32 cols 40:56  onesw as f16[32]: ones at f16-col 15 (sliding lhsT)
    #   f32 cols 64:128 vwin: 4 chunks of 32 f16 cols, chunk o has
    #                   v[o*128+p] at f16-col 128+32*o+15 (sliding lhsT)
    #   f32 cols 128:256  wet0 as fp8 [k*128+o]: the o=0 slice of wetp,
    #                   wet0[p, k, o] = fp8(SW * W_e^T[k*128+p, o]) (o<128)
    consts = nc.declare_dram_parameter("consts", [P, 256], F32, isOutput=False)
    out = nc.declare_dram_parameter("out", [NB, T], F32, isOutput=True)

    with tile.TileContext(nc) as tc, ExitStack() as ctx:
        const_pool = ctx.enter_context(tc.tile_pool(name="const", bufs=1))
        enc_pool = ctx.enter_context(tc.tile_pool(name="enc", bufs=1))
        tanh_pool = ctx.enter_context(tc.tile_pool(name="tanh", bufs=6))
        sm_pool = ctx.enter_context(tc.tile_pool(name="sm", bufs=1))
        psE_pool = ctx.enter_context(tc.tile_pool(name="psE", bufs=3, space="PSUM"))
        psS_pool = ctx.enter_context(tc.tile_pool(name="psS", bufs=1, space="PSUM"))

        def load_sup(s, engine=None):
            e = enc_pool.tile([P, KC, SUP], F8, tag=f"e{s}", name=f"e{s}")
            (engine or nc.sync).dma_start(
                e[:], enct[:, :, s * SUP : (s + 1) * SUP]
            )
            return e

        # whole enc shard stays SBUF-resident (4 * 4KB/partition = 16KB);
        # one 512KB DMA per superblock. The first superblock is split per
        # k-chunk across BOTH rings (kp0 on sync, kp1 on scalar behind
        # the small wet0) so both DoubleRow k-pairs land in ~2 DMA slots
        # and the (s0, o0) group closes ~4us earlier. The tiny consts
        # block leads the sync ring so the q bias beats the first tanh.
        enc_sb = [None] * NSUP
        # the consts+wet0 block leads the sync ring ahead of e0's kp0;
        # each e0 k-PAIR moves as ONE strided DMA (ring completions
        # pipeline ~1.2us apart, so one 256KB transfer beats two 128KB
        # slots), kp0 on sync and kp1 leading the scalar ring
        c_sb = const_pool.tile([P, 256], F32, tag="consts")
        nc.sync.dma_start(c_sb[:], consts[:, :])
        e0 = enc_pool.tile([P, KC, SUP], F8, tag="e0", name="e0")
        # kp0 leads the scalar ring (the accumulation group opens with
        # kp0, so its matmuls can start while kp1 rides sync slot 2
        # behind consts)
        nc.scalar.dma_start(e0[:, 0:2, :], enct[:, 0:2, 0:SUP])
        nc.sync.dma_start(e0[:, 2:KC, :], enct[:, 2:KC, 0:SUP])
        enc_sb[0] = e0
        c16 = c_sb[:].bitcast(F16)  # (P, 512) f16 view
        q_sb = c_sb[:, 0:16]
        vp_sb = c_sb[:, 16:20]
        m16_sb = c_sb[0:16, 20:36]
        onesw_sb = c16[:, 80:111]
        vwin_sb = [c16[:, 128 + 32 * o : 159 + 32 * o] for o in range(OC)]
        wet0_sb = (
            c_sb[:, 128:256].bitcast(F8).rearrange("p (k o) -> p k o", k=KC)
        )

        wetp_sb = const_pool.tile([P, KC, H - P], F8, tag="wetp")
        nc.scalar.dma_start(wetp_sb[:], wetp[:, :, :])

        # late superblocks ride the scalar ring (it is idle after the
        # constant loads), halving the enc DMA tail on the sync ring
        for s in range(1, NSUP):
            enc_sb[s] = load_sup(
                s, engine=nc.scalar if s >= NSUP // 2 else nc.sync
            )

        # PE warmup: 8 N=512 matmuls on a zeroed tile fill the ~3us
        # head-DMA shadow with enough sustained activity to release the
        # HAM clock gate (2.4 GHz) before the first real matmuls.
        warm = const_pool.tile([P, BLK], F16, tag="warm")
        nc.gpsimd.memset(warm[:], 0.0)
        psW = psS_pool.tile([P, BLK], F32, tag="t", name="psW")
        for _ in range(8):
            nc.tensor.matmul(
                psW[:], lhsT=warm[:, 0:P], rhs=warm[:],
                start=True, stop=True,
            )

        # single persistent score accumulator: row 2s+h2 = scores of
        # (batch s//2, t-slice (s%2)*1024 + h2*512)
        psS16 = psS_pool.tile([16, BLK], F32, tag="s16", name="psS16")

        # z(s) = sum_o v_o * tanh(E_o + q_o) is built on DVE (fused
        # scalar_tensor_tensor), then one small matmul per 512-block
        # contracts the 128 partitions into psS. The z-matmul for s is
        # emitted during s+1's main matmuls so the in-order PE never
        # waits on the DVE chain it just scheduled.
        def emit_zmm(z, s, b):
            for h2 in range(SUP // BLK):
                c = 2 * s + h2
                nc.tensor.matmul(
                    psS16[:],
                    lhsT=onesw_sb[:, 15 - c : 31 - c],
                    rhs=z[:, h2 * BLK : (h2 + 1) * BLK],
                    start=(s == 0 and h2 == 0),
                    stop=(s == NSUP - 1 and h2 == 1),
                )

        def emit_vdot(th, s, o):
            # direct PE v-dot (used for the last superblock so the tail
            # does not wait on the DVE z-chain)
            for h2 in range(SUP // BLK):
                c = 2 * s + h2
                nc.tensor.matmul(
                    psS16[:],
                    lhsT=vwin_sb[o][:, 15 - c : 31 - c],
                    rhs=th[:, h2 * BLK : (h2 + 1) * BLK],
                    start=False,
                    stop=(s == NSUP - 1 and o == OC - 1 and h2 == 1),
                )

        # Tile schedule: each superblock's o0 is hoisted one slot early
        # ((s,3) runs after (s+1,0)) so the ACT engine rolls straight
        # from sup s into sup s+1 without waiting for s+1's first mains
        # at the boundary — PE produces E(s+1,o0) one ACT-slot ahead.
        sched = [(0, 0), (0, 1), (0, 2)]
        for s in range(NSUP - 1):
            sched += [(s + 1, 0), (s, 3), (s + 1, 1), (s + 1, 2)]
        sched += [(NSUP - 1, 3)]

        pending = None
        zt = {}          # live z tile per superblock
        th7 = {}         # last-superblock tanh tiles for the PE v-dot
        for s, o in sched:
            b = s // 2
            last_sup = s == NSUP - 1
            psE = psE_pool.tile([P, SUP], F32, tag="E")
            th = tanh_pool.tile([P, SUP], F16, tag="tanh")

            def lhsT_for(kp, o=o):
                return (
                    wet0_sb[:, 2 * kp : 2 * kp + 2, :]
                    if o == 0
                    else wetp_sb[:, 2 * kp : 2 * kp + 2, (o - 1) * P : o * P]
                )

            # kp-outer / h2-inner: each LDWEIGHTS feeds two
            # N=512 matmuls (PSUM caps the output at one bank)
            for kp in range(KP):
                for h2 in range(SUP // BLK):
                    nc.tensor.matmul(
                        psE[:, h2 * BLK : (h2 + 1) * BLK],
                        lhsT=lhsT_for(kp),
                        rhs=enc_sb[s][
                            :,
                            2 * kp : 2 * kp + 2,
                            h2 * BLK : (h2 + 1) * BLK,
                        ],
                        start=(kp == 0),
                        stop=(kp == KP - 1),
                        perf_mode=DR,
                    )
            nc.scalar.activation(
                th[:],
                psE[:],
                AF.Tanh,
                bias=q_sb[:, o * NB + b : o * NB + b + 1],
                scale=1.0 / (SE * SW),
            )
            if not last_sup:
                if o == 0:
                    zt[s] = tanh_pool.tile(
                        [P, SUP], F16, tag="z", bufs=3, name=f"z{s}"
                    )
                    nc.vector.tensor_scalar_mul(zt[s][:], th[:], vp_sb[:, 0:1])
                else:
                    nc.vector.scalar_tensor_tensor(
                        zt[s][:],
                        th[:],
                        vp_sb[:, o : o + 1],
                        zt[s][:],
                        op0=mybir.AluOpType.mult,
                        op1=mybir.AluOpType.add,
                    )
                if o == 3:
                    pending = (zt.pop(s), s, b)
            else:
                # defer ALL last-superblock v-dots until after its final
                # mains: ACT(7,o3) then gates only the 2-matmul stop pair
                # before the softmax exp can fire
                th7[o] = th
            if pending is not None and o == 1:
                emit_zmm(*pending)
                pending = None
        for o in range(OC):
            emit_vdot(th7.pop(o), NSUP - 1, o)

        # softmax on the (16, 512) layout; scores are bounded (|s| <=
        # ||v||_1) so no max-subtraction is needed in f32
        ex16 = sm_pool.tile([16, BLK], F32, tag="ex16")
        sums16 = sm_pool.tile([16, 1], F32, tag="sums16")
        nc.scalar.activation(ex16[:], psS16[:], AF.Exp, accum_out=sums16[:])
        # one block-diagonal (16,16) matmul fuses contract-per-batch and
        # broadcast-back-to-rows; then a single reciprocal feeds the mult
        psB = psS_pool.tile([16, 1], F32, tag="t", name="psB")
        nc.tensor.matmul(
            psB[:], lhsT=m16_sb[:], rhs=sums16[:], start=True, stop=True
        )
        rec16 = sm_pool.tile([16, 1], F32, tag="rec16")
        nc.vector.reciprocal(rec16[:], psB[:])
        probs16 = sm_pool.tile([16, BLK], F32, tag="probs16")
        nc.vector.tensor_scalar_mul(probs16[:], ex16[:], rec16[:, 0:1])
        nc.sync.dma_start(
            out[:, :].rearrange("b (j t) -> (b j) t", j=4), probs16[:]
        )

    nc.compile()
    return nc


_NC = None


def _get_nc():
    global _NC
    if _NC is None:
        _NC = _build()
    return _NC


def _shard_inputs(hidden, encoder_outputs, W_attn, b_attn, v):
    hidden = np.asarray(hidden, dtype=np.float32)
    encoder_outputs = np.asarray(encoder_outputs, dtype=np.float32)
    W_attn = np.asarray(W_attn, dtype=np.float32)
    b_attn = np.asarray(b_attn, dtype=np.float32)
    v = np.asarray(v, dtype=np.float32)

    # wetp[p, k, o-128] = fp8(SW * W_e^T[k*128+p, o]) for o >= 128
    wet_t = (W_attn[:, H:].T * SW).astype(NP8)  # (H, H) [h, o]
    wetp = np.ascontiguousarray(
        wet_t.reshape(KC, P, H)[:, :, P:].transpose(1, 0, 2)
    )
    wet0 = np.ascontiguousarray(
        wet_t.reshape(KC, P, H)[:, :, :P].transpose(1, 0, 2)
    )

    # q computed exactly on the host: (B, H)
    q_full = (
        hidden[0].astype(np.float64) @ W_attn[:, :H].astype(np.float64).T
        + b_attn.astype(np.float64)
    ).astype(np.float32)

    # packed constant block, f32 view (P, 256) / f16 view (P, 512)
    consts = np.zeros((P, 256), dtype=np.float32)
    c16 = consts.view(np.float16)  # (P, 512)
    consts[:, 16:20] = v.reshape(OC, P).T
    for i in range(16):
        for j in range(16):
            if i // NB == j // NB:
                consts[i, 20 + j] = 1.0  # m16: block-diagonal per batch
    c16[:, 80 + 15] = np.float16(1.0)  # onesw: ones at f16-col 15
    vrT = v.reshape(OC, P).T.astype(np.float16)  # (P, OC)
    for o in range(OC):
        c16[:, 128 + 32 * o + 15] = vrT[:, o]  # vwin sliding windows
    # wet0 (fp8, (P, KC*P) bytes) rides in f32 cols 128:256
    consts[:, 128:256] = np.ascontiguousarray(
        wet0.reshape(P, KC * P)
    ).view(np.float32)

    # (H, B, T) so per-core slices are cheap views before the copy
    enc_hbt = np.transpose(encoder_outputs, (2, 1, 0))
    enc_all8 = (enc_hbt * SE).astype(NP8)  # one fp8 cast for all cores
    in_maps = []
    for c in range(NCORES):
        b0 = c * NB
        # enct[p, k, r] = fp8(SE * enc^T[k*128+p, r])
        enct = np.ascontiguousarray(
            enc_all8[:, b0 : b0 + NB, :]
        ).reshape(KC, P, ROWS)
        enct = np.ascontiguousarray(enct.transpose(1, 0, 2))
        cc = consts.copy()
        # qrep[p, o*NB+b] = q[b0+b, o*128+p]
        cc[:, 0:16] = (
            q_full[b0 : b0 + NB, :].T.reshape(OC, P, NB)
            .transpose(1, 0, 2).reshape(P, OC * NB)
        )
        in_maps.append(
            {
                "enct": enct,
                "wetp": wetp,
                "consts": cc,
            }
        )
    return in_maps


def kernel(hidden, encoder_outputs, W_attn, b_attn, v):
    nc = _get_nc()
    in_maps = _shard_inputs(hidden, encoder_outputs, W_attn, b_attn, v)
    res = bass_utils.run_bass_kernel_spmd(
        nc, in_maps, core_ids=list(range(NCORES))
    )
    outs = [res.results[c]["out"] for c in range(NCORES)]  # each (NB, T)
    full = np.concatenate(outs, axis=0)  # (B, T)
    return full[:, None, :].astype(np.float32)  # (B, 1, T)

